# revision 9
# baseline (speedup 1.0000x reference)
# Block-local matmul kernel for Trainium2 (8 NeuronCores, SPMD) — v2.
#
# Problem: out[b, i*64+r, j*64+o] = sum_c x[b, i*64+r, j*64+c] * W[i*64+c, j*64+o]
# with B=4, M=K=N=4096, 64x64 blocks. Embarrassingly parallel over (i, j).
#
# Sharding: block-row axis i across the 8 cores. Core p gets rows
# [512p, 512p+512) of x/out and rows [512p, 512p+512) of weight. No
# collectives; outputs are reassembled on the host.
#
# v2 design (v1 was sequencer-bound: ~3k PE instructions x ~71ns decode
# dominated its 217us):
#   - x is cast to bf16 on the host (halves x HBM read traffic) and loaded
#     with HWDGE DMA-transpose (xbar): A^T lands directly in SBUF with
#     partition = contraction index. Eliminates all 512 PE transposes and
#     the PSUM->SBUF transpose copies of v1.
#   - W is prebuilt on the host as block-diagonal j-pair tiles wd[c2, i, s,
#     o2] (c2 = stacked c of j=2s / j=2s+1, o2 = stacked out cols): one
#     matmul per j-PAIR with full 128-deep contraction -> 512 matmuls/core,
#     each writing a contiguous [128, 128] PSUM region.
#   - Output is stored as bf16 in strip layout [u, i, (t,r), n] (one clean
#     2D [128, 4096] DMA per strip) and upcast/reassembled on the host.
#   - PSUM: one matmul group per 2KB bank (HW faults if a bank mixes
#     groups and is then read); gather copies read only the written
#     [*, q, 0:128] slices of 4 banks at a time, split DVE/ACT.
#
# Per-core HBM traffic: 16MB x + 8MB wd + 16MB out = 40MB -> ~112us at
# the ~358 GB/s per-core HBM limit. PE ~512*71ns seq + ~27us stream.

import numpy as np

B = 4
M = K = N = 4096
NCORES = 8
RPC = M // NCORES  # 512 rows per core
NI = RPC // 64     # 8 i-blocks per core
NJ = N // 64       # 64 j-blocks
NS = NJ // 2       # 32 j-pairs

_NC_CACHE = None


def _build_nc():
    import concourse.tile as tile
    from concourse import bacc, mybir

    f32 = mybir.dt.float32
    bf16 = mybir.dt.bfloat16

    nc = bacc.Bacc("TRN2", target_bir_lowering=False, debug=False,
                   num_devices=NCORES)
    # x prearranged on host to strip layout [u, i, (t, r), K] so each
    # strip is a contiguous 2D [128, 4096] source for one transpose DMA.
    x_d = nc.dram_tensor("x_shard", [2, NI, 128, K], bf16,
                         kind="ExternalInput")
    wd_d = nc.dram_tensor("wd_shard", [128, NI, NS, 128], bf16,
                          kind="ExternalInput")
    o_d = nc.dram_tensor("out_shard", [2, NI, 128, N], bf16,
                         kind="ExternalOutput")

    with tile.TileContext(nc) as tc:
        with (
            tc.tile_pool(name="wd", bufs=1) as wdp,
            tc.tile_pool(name="at", bufs=3) as atp,
            tc.tile_pool(name="ob", bufs=3) as obp,
            tc.tile_pool(name="psO", bufs=2, space="PSUM") as psOp,
        ):
            # Block-diag W, loaded in 8 per-i pieces so the first strip's
            # matmuls only wait for piece 0.
            wd = wdp.tile([128, NI, NS, 128], bf16)
            for i in range(NI):
                nc.gpsimd.dma_start(wd[:, i, :, :], wd_d.ap()[:, i, :, :])

            for u in range(2):        # batch pair (b in {2u, 2u+1})
                for i in range(NI):   # i-block within core
                    # A^T via one DMA transpose per strip:
                    # atb[c2, s, tr] = x_strip[tr, 128s+c2]. The dst must
                    # be CONTIGUOUS (the xbar writes wrong offsets into a
                    # strided mid-dim dst — HW-verified) and the matmul
                    # stationary AP allows only ONE free dim, hence the
                    # host-side strip prearrangement of x.
                    atb = atp.tile([128, NS, 128], bf16, tag="at")
                    eng = nc.sync if (u * NI + i) % 2 == 0 else nc.scalar
                    eng.dma_start_transpose(atb[:], x_d.ap()[u, i])

                    ob = obp.tile([128, N], bf16, tag="ob")
                    for g in range(8):           # groups of 4 j-pairs
                        psO = psOp.tile([128, 4, 512], f32, tag="psO")
                        for q in range(4):
                            s = 4 * g + q
                            nc.tensor.matmul(
                                psO[:, q, 0:128], atb[:, s, :],
                                wd[:, i, s, :], start=True, stop=True)
                        dst = ob[:, 512 * g:512 * g + 512]
                        dst = dst.rearrange("p (q o) -> p q o", q=4)
                        if g % 2 == 0:
                            nc.vector.tensor_copy(dst, psO[:, :, 0:128])
                        else:
                            nc.scalar.copy(dst, psO[:, :, 0:128])

                    # One clean 2D [128, 4096] store on the other ring.
                    seng = nc.scalar if (u * NI + i) % 2 == 0 else nc.sync
                    seng.dma_start(o_d.ap()[u, i], ob[:])

    nc.compile()
    return nc


def _get_nc():
    global _NC_CACHE
    if _NC_CACHE is None:
        _NC_CACHE = _build_nc()
    return _NC_CACHE


def prepare(x, weight):
    """Build (cached) nc and per-core input maps from full inputs."""
    import ml_dtypes

    bf16 = ml_dtypes.bfloat16
    x = np.asarray(x, dtype=np.float32)
    w = np.asarray(weight, dtype=np.float32)
    assert x.shape == (B, M, K) and w.shape == (K, N)
    x16 = x.astype(bf16)
    w16 = w.astype(bf16)

    nc = _get_nc()
    in_maps = []
    for c in range(NCORES):
        rows = slice(RPC * c, RPC * (c + 1))
        # Block-diag j-pair W: wd[c2, i, s, o2]; quad (0,0) = W(i, 2s),
        # quad (1,1) = W(i, 2s+1), off-diagonal quads zero.
        wc = w16[rows].reshape(NI, 64, NS, 2, 64)
        wd = np.zeros((128, NI, NS, 128), dtype=bf16)
        wd[0:64, :, :, 0:64] = wc[:, :, :, 0, :].transpose(1, 0, 2, 3)
        wd[64:128, :, :, 64:128] = wc[:, :, :, 1, :].transpose(1, 0, 2, 3)
        # Strip layout [u, i, (t, r), K]: b = 2u + t.
        xs = (x16[:, rows, :].reshape(2, 2, NI, 64, K)
              .transpose(0, 2, 1, 3, 4).reshape(2, NI, 128, K))
        in_maps.append({
            "x_shard": np.ascontiguousarray(xs),
            "wd_shard": wd,
        })
    return nc, in_maps


def kernel(x, weight):
    from concourse import bass_utils

    nc, in_maps = prepare(x, weight)
    res = bass_utils.run_bass_kernel_spmd(nc, in_maps,
                                          core_ids=list(range(NCORES)))
    out = np.empty((B, M, N), dtype=np.float32)
    for c in range(NCORES):
        # out_shard[u, i, (t, r), n] -> out[2u+t, 512c + 64i + r, n]
        arr = res.results[c]["out_shard"].reshape(2, NI, 2, 64, N)
        out[:, RPC * c:RPC * (c + 1), :] = (
            arr.transpose(0, 2, 1, 3, 4).reshape(B, RPC, N))
    return out
